# revision 17
# baseline (speedup 1.0000x reference)
"""AttentionBlock Trainium2 kernel.

Reference computation (per batch b):
    xf = x[b].reshape(N, C);  N = 64*64 = 4096, C = 256, d = C//8 = 32
    q = xf @ Wq + bq; k = xf @ Wk + bk; v = xf @ Wv + bv
    out = softmax(q @ k.T) @ v
    y = gamma * out + xf

Sharding: 8 cores = 4 batches x 2 halves of the query rows. Each core
computes k/v for its full batch and attention for its 2048 query rows.

Per-core kernel layout choices:
  - Host passes xT (x[b] transposed, own query half rolled to the front) so
    all projection matmuls contract over channels on the partition dim.
  - q/k are projected with 4x-replicated weights (Wq tiled to [256,128]) so
    the d=32 contraction of the score matmul can be row-packed 4 ways
    (tile_position) and fill the whole 128x128 PE array.
  - Scores are computed TRANSPOSED (scoresT[m, n] = k[m].q[n]) so that after
    exp, the attention weights are already in the right layout to be the
    stationary operand of the attn@v matmul, with output in natural [n, c]
    layout - no transposes anywhere.
  - v is augmented with a ones column, so the attn@v accumulation also
    produces the softmax denominator (column 256) for free.
  - All matmuls use float32r (full-rate fp32 mode on the PE array).
"""

import numpy as np

CH = 256
DQK = 32
N = 4096  # H*W
NQ = 2048  # query rows per core
B = 4
N_CORES = 8
CH2 = CH + 2  # v augmented with [denominator-ones, pad] columns (fp32r needs even)
CBLOB = 1418  # packed constants blob width (see _pack_consts)

_COMPILED = {}


def _build():
    """Build + compile the single-program SPMD Bass kernel. Cached."""
    if "nc" in _COMPILED:
        return _COMPILED["nc"]

    import concourse.bass as bass
    import concourse.tile as tile
    from concourse import bacc, mybir

    f32 = mybir.dt.float32
    f32r = mybir.dt.float32r
    AF = mybir.ActivationFunctionType
    OP = mybir.AluOpType

    nc = bacc.Bacc(
        "TRN2",
        target_bir_lowering=False,
        debug=False,
        enable_asserts=True,
        num_devices=N_CORES,
    )

    # ---- I/O ----
    xT = nc.dram_tensor("xT", [CH, N], f32, kind="ExternalInput").ap()
    xres = nc.dram_tensor("xres", [NQ, CH], f32, kind="ExternalInput").ap()
    # all small constants packed into one blob (single DMA); see _pack_consts
    cblob_d = nc.dram_tensor("cblob", [128, CBLOB], f32, kind="ExternalInput").ap()
    y = nc.dram_tensor("y", [NQ, CH], f32, kind="ExternalOutput").ap()

    MT = N // 128  # 32 key tiles
    NS = NQ // 512  # 4 query slices
    NGRP = MT // 4  # 8 groups of 4 key tiles

    with tile.TileContext(nc) as tc:
        with (
            tc.tile_pool(name="consts", bufs=1) as consts,
            tc.tile_pool(name="xtp", bufs=1) as xtp,
            tc.tile_pool(name="qk", bufs=1) as qkp,
            tc.tile_pool(name="vp", bufs=1) as vp,
            tc.tile_pool(name="xrp", bufs=1) as xrp,
            tc.tile_pool(name="expp", bufs=3) as expp,
            tc.tile_pool(name="yp", bufs=4) as yp,
            tc.tile_pool(name="smallp", bufs=8) as smallp,
        ):
            # ---- constants (one DMA) + x loads spread across DMA queues ----
            cb = consts.tile([128, CBLOB], f32r)
            nc.scalar.dma_start(cb[:], cblob_d[:, :].bitcast(f32r))
            # views into the blob (layout must match _pack_consts)
            wq4s = lambda kt: cb[:, 128 * kt : 128 * (kt + 1)]
            wk4s = lambda kt: cb[:, 256 + 128 * kt : 256 + 128 * (kt + 1)]
            wvs = lambda kt: cb[:, 512 + CH2 * kt : 512 + CH2 * (kt + 1)]
            bq4s = cb[:, 1028:1029].bitcast(f32)
            bk4s = cb[:, 1029:1030].bitcast(f32)
            bvs = cb[0:1, 1030 : 1030 + CH2]
            gs = cb[0:1, 1288:1290]
            oness = cb[0:1, 1290:1418]

            xts = xtp.tile([128, 2, N], f32r)
            xTr = xT.rearrange("(t p) n -> p t n", p=128)
            # (kt, h) -> engine; q/k matmuls need h=0 chunks of both kt first
            dma_engines = {
                (0, 0): nc.sync,
                (1, 0): nc.scalar,
                (0, 1): nc.gpsimd,
                (1, 1): nc.sync,
            }
            for (kt, h) in [(0, 0), (1, 0), (0, 1), (1, 1)]:
                dma_engines[(kt, h)].dma_start(
                    xts[:, kt, 2048 * h : 2048 * (h + 1)],
                    xTr[:, kt, 2048 * h : 2048 * (h + 1)].bitcast(f32r),
                )

            xr = xrp.tile([128, NQ // 128, CH], f32)
            nc.gpsimd.dma_start(xr[:], xres.rearrange("(t p) c -> p t c", p=128))

            qt4 = qkp.tile([128, NQ], f32r)
            kt4 = qkp.tile([128, N], f32r)
            vaug = vp.tile([128, MT, CH2], f32r)

            # ---- broadcasts (bias row, gamma) via K=1 outer-product matmuls ----
            with tc.tile_pool(name="psb", bufs=2, space="PSUM") as psb:
                pb = psb.tile([128, CH2], f32)
                nc.tensor.matmul(
                    pb[:],
                    lhsT=oness.bitcast(f32r),
                    rhs=bvs.bitcast(f32r),
                    start=True,
                    stop=True,
                )
                bvb = consts.tile([128, CH2], f32)
                nc.vector.tensor_copy(bvb[:], pb[:])

                pg = psb.tile([128, 2], f32)
                nc.tensor.matmul(
                    pg[:],
                    lhsT=oness.bitcast(f32r),
                    rhs=gs.bitcast(f32r),
                    start=True,
                    stop=True,
                )
                gb = consts.tile([128, 2], f32)
                nc.vector.tensor_copy(gb[:], pg[:])

            # ---- projections ----
            with (
                tc.tile_pool(name="psqk", bufs=2, space="PSUM") as psqk,
                tc.tile_pool(name="psv", bufs=4, space="PSUM") as psv,
            ):
                # qT4[32a+d, n] = q[n, d] (own half), replicated over a
                for t in range(NS):
                    pq = psqk.tile([128, 512], f32)
                    for kt in range(2):
                        nc.tensor.matmul(
                            pq[:],
                            lhsT=wq4s(kt).bitcast(f32r),
                            rhs=xts[:, kt, 512 * t : 512 * (t + 1)].bitcast(f32r),
                            start=(kt == 0),
                            stop=(kt == 1),
                        )
                    nc.vector.tensor_scalar_add(
                        qt4[:, 512 * t : 512 * (t + 1)], pq[:], bq4s
                    )
                # kT4 over the full batch
                for t in range(N // 512):
                    pk = psqk.tile([128, 512], f32)
                    for kt in range(2):
                        nc.tensor.matmul(
                            pk[:],
                            lhsT=wk4s(kt).bitcast(f32r),
                            rhs=xts[:, kt, 512 * t : 512 * (t + 1)].bitcast(f32r),
                            start=(kt == 0),
                            stop=(kt == 1),
                        )
                    nc.vector.tensor_scalar_add(
                        kt4[:, 512 * t : 512 * (t + 1)], pk[:], bk4s
                    )
                # v_aug[m, 0:256] = v natural; v_aug[m, 256] = 1 (bias col)
                for mt in range(MT):
                    pv = psv.tile([128, CH2], f32)
                    for kt in range(2):
                        nc.tensor.matmul(
                            pv[:],
                            lhsT=xts[:, kt, 128 * mt : 128 * (mt + 1)].bitcast(f32r),
                            rhs=wvs(kt).bitcast(f32r),
                            start=(kt == 0),
                            stop=(kt == 1),
                        )
                    nc.vector.tensor_tensor(
                        vaug[:, mt, :], pv[:], bvb[:], op=OP.add
                    )

            # ---- attention main loop ----
            # Groups of 2 key tiles; score PSUM double-buffered (2+2 banks)
            # so the PE can prefill the next group's scores while the
            # ScalarE exps the current one. Consecutive groups use disjoint
            # PE row-strip pairs so their packed matmuls overlap in the
            # array as well.
            NG2 = MT // 2  # 16 groups per n-slice
            with (
                tc.tile_pool(name="pss", bufs=2, space="PSUM") as pss,
                tc.tile_pool(name="psa", bufs=1, space="PSUM") as psa,
            ):
                def scores_mm(ns, g, s):
                    # scoresT[m, n] for 2 key tiles, row-packed (K=32 x2);
                    # alternating strip pairs let consecutive groups overlap
                    for i in range(2):
                        mt = 2 * g + i
                        st = (2 * g + i) % 4  # row strip
                        nc.tensor.matmul(
                            s[:, i, :],
                            lhsT=kt4[
                                32 * st : 32 * (st + 1), 128 * mt : 128 * (mt + 1)
                            ].bitcast(f32r),
                            rhs=qt4[
                                32 * st : 32 * (st + 1), 512 * ns : 512 * (ns + 1)
                            ].bitcast(f32r),
                            start=True,
                            stop=True,
                            tile_position=(32 * st, 0),
                        )

                groups = [(ns, g) for ns in range(NS) for g in range(NG2)]
                acc = None
                s_cur = pss.tile([128, 2, 512], f32)
                scores_mm(*groups[0], s_cur)
                for idx, (ns, g) in enumerate(groups):
                    if g == 0:
                        acc = psa.tile([128, 4, 512], f32)
                    e = expp.tile([128, 2, 512], f32r)
                    nc.scalar.activation(e[:], s_cur[:], AF.Exp)
                    # emit the next group's scores BEFORE this group's accum
                    # so the PE unblocks the ScalarE first
                    if idx + 1 < len(groups):
                        s_cur = pss.tile([128, 2, 512], f32)
                        scores_mm(*groups[idx + 1], s_cur)
                    # acc[n, :] += expT[m, n].T-as-weights @ v_aug[m, :]
                    for i in range(2):
                        mt = 2 * g + i
                        for j in range(4):
                            nc.tensor.matmul(
                                acc[:, j, 0:CH2],
                                lhsT=e[:, i, 128 * j : 128 * (j + 1)].bitcast(f32r),
                                rhs=vaug[:, mt, :].bitcast(f32r),
                                start=(g == 0 and i == 0),
                                stop=(g == NG2 - 1 and i == 1),
                            )
                    if g == NG2 - 1:
                        # normalize + gamma + residual, store
                        for j in range(4):
                            nt = 4 * ns + j
                            r = smallp.tile([128, 1], f32)
                            nc.vector.reciprocal(r[:], acc[:, j, CH : CH + 1])
                            rg = smallp.tile([128, 1], f32)
                            nc.vector.tensor_tensor(rg[:], r[:], gb[:, 0:1], op=OP.mult)
                            yt = yp.tile([128, CH], f32)
                            nc.vector.scalar_tensor_tensor(
                                yt[:],
                                acc[:, j, 0:CH],
                                rg[:, 0:1],
                                xr[:, nt, :],
                                op0=OP.mult,
                                op1=OP.add,
                            )
                            nc.sync.dma_start(y[128 * nt : 128 * (nt + 1), :], yt[:])

    nc.compile()
    _COMPILED["nc"] = nc
    return nc


def _pack_consts(Wq, bq, Wk, bk, Wv, bv, gamma):
    """Pack all small constants into one [128, CBLOB] blob.

    Layout (per partition p):
      [0:256)     Wq4 k-tiles: [wq4[p], wq4[p+128]]   (wq4 = tile(Wq, (1,4)))
      [256:512)   Wk4 k-tiles
      [512:1028)  Wv_aug k-tiles (CH2 = 258 each)
      [1028]      bq4[p];  [1029] bk4[p]
      partition 0 only:
      [1030:1288) bv_aug (bv ++ [1.0, 0.0])
      [1288:1290) gamma, 0
      [1290:1418) ones
    """
    Wq4 = np.tile(np.asarray(Wq, np.float32), (1, 4))  # [256, 128]
    Wk4 = np.tile(np.asarray(Wk, np.float32), (1, 4))
    bq4 = np.tile(np.asarray(bq, np.float32), 4)  # [128]
    bk4 = np.tile(np.asarray(bk, np.float32), 4)
    Wv_aug = np.zeros((CH, CH2), np.float32)
    Wv_aug[:, :CH] = np.asarray(Wv, np.float32)

    cb = np.zeros((128, CBLOB), np.float32)
    for kt in range(2):
        cb[:, 128 * kt : 128 * (kt + 1)] = Wq4[128 * kt : 128 * (kt + 1), :]
        cb[:, 256 + 128 * kt : 256 + 128 * (kt + 1)] = Wk4[128 * kt : 128 * (kt + 1)]
        cb[:, 512 + CH2 * kt : 512 + CH2 * (kt + 1)] = Wv_aug[
            128 * kt : 128 * (kt + 1), :
        ]
    cb[:, 1028] = bq4
    cb[:, 1029] = bk4
    cb[0, 1030 : 1030 + CH] = np.asarray(bv, np.float32)
    cb[0, 1030 + CH] = 1.0
    cb[0, 1288] = np.float32(np.asarray(gamma).reshape(()))
    cb[0, 1290:1418] = 1.0
    return cb


def _shard_inputs(x, Wq, bq, Wk, bk, Wv, bv, gamma):
    """Host-side prep: one input map per core."""
    xf = np.ascontiguousarray(x, dtype=np.float32).reshape(B, N, CH)
    cb = _pack_consts(Wq, bq, Wk, bk, Wv, bv, gamma)

    in_maps = []
    for c in range(N_CORES):
        b, h = divmod(c, 2)
        own = slice(h * NQ, (h + 1) * NQ)
        other = slice((1 - h) * NQ, (2 - h) * NQ)
        xT_b = xf[b].T  # [CH, N]
        xT_roll = np.ascontiguousarray(
            np.concatenate([xT_b[:, own], xT_b[:, other]], axis=1)
        )
        in_maps.append(
            {
                "xT": xT_roll,
                "xres": np.ascontiguousarray(xf[b, own]),
                "cblob": cb,
            }
        )
    return in_maps


def kernel(x, Wq, bq, Wk, bk, Wv, bv, gamma):
    from concourse.bass_utils import run_bass_kernel_spmd

    nc = _build()
    in_maps = _shard_inputs(x, Wq, bq, Wk, bk, Wv, bv, gamma)
    res = run_bass_kernel_spmd(nc, in_maps, core_ids=list(range(N_CORES)))
    out = np.empty((B, N, CH), np.float32)
    for c in range(N_CORES):
        b, h = divmod(c, 2)
        out[b, h * NQ : (h + 1) * NQ, :] = res.results[c]["y"]
    return out.reshape(x.shape)


# revision 18
# speedup vs baseline: 1.0329x; 1.0329x over previous
"""AttentionBlock Trainium2 kernel.

Reference computation (per batch b):
    xf = x[b].reshape(N, C);  N = 64*64 = 4096, C = 256, d = C//8 = 32
    q = xf @ Wq + bq; k = xf @ Wk + bk; v = xf @ Wv + bv
    out = softmax(q @ k.T) @ v
    y = gamma * out + xf

Sharding: 8 cores = 4 batches x 2 halves of the query rows. Each core
computes k/v for its full batch and attention for its 2048 query rows.

Per-core kernel layout choices:
  - Host passes xT (x[b] transposed, own query half rolled to the front) so
    all projection matmuls contract over channels on the partition dim.
  - q/k are projected with 4x-replicated weights (Wq tiled to [256,128]) so
    the d=32 contraction of the score matmul can be row-packed 4 ways
    (tile_position) and fill the whole 128x128 PE array.
  - Scores are computed TRANSPOSED (scoresT[m, n] = k[m].q[n]) so that after
    exp, the attention weights are already in the right layout to be the
    stationary operand of the attn@v matmul, with output in natural [n, c]
    layout - no transposes anywhere.
  - v is augmented with a ones column, so the attn@v accumulation also
    produces the softmax denominator (column 256) for free.
  - All matmuls use float32r (full-rate fp32 mode on the PE array).
"""

import numpy as np

CH = 256
DQK = 32
N = 4096  # H*W
NQ = 2048  # query rows per core
B = 4
N_CORES = 8
CH2 = CH + 2  # v augmented with [denominator-ones, pad] columns (fp32r needs even)
CBLOB = 1418  # packed constants blob width (see _pack_consts)

_COMPILED = {}


def _build():
    """Build + compile the single-program SPMD Bass kernel. Cached."""
    if "nc" in _COMPILED:
        return _COMPILED["nc"]

    import concourse.bass as bass
    import concourse.tile as tile
    from concourse import bacc, mybir

    f32 = mybir.dt.float32
    f32r = mybir.dt.float32r
    AF = mybir.ActivationFunctionType
    OP = mybir.AluOpType

    nc = bacc.Bacc(
        "TRN2",
        target_bir_lowering=False,
        debug=False,
        enable_asserts=True,
        num_devices=N_CORES,
    )

    # ---- I/O ----
    xT = nc.dram_tensor("xT", [CH, N], f32, kind="ExternalInput").ap()
    xres = nc.dram_tensor("xres", [NQ, CH], f32, kind="ExternalInput").ap()
    # all small constants packed into one blob (single DMA); see _pack_consts
    cblob_d = nc.dram_tensor("cblob", [128, CBLOB], f32, kind="ExternalInput").ap()
    y = nc.dram_tensor("y", [NQ, CH], f32, kind="ExternalOutput").ap()

    MT = N // 128  # 32 key tiles
    NS = NQ // 512  # 4 query slices
    NGRP = MT // 4  # 8 groups of 4 key tiles

    with tile.TileContext(nc) as tc:
        with (
            tc.tile_pool(name="consts", bufs=1) as consts,
            tc.tile_pool(name="xtp", bufs=1) as xtp,
            tc.tile_pool(name="qk", bufs=1) as qkp,
            tc.tile_pool(name="vp", bufs=1) as vp,
            tc.tile_pool(name="xrp", bufs=1) as xrp,
            tc.tile_pool(name="expp", bufs=3) as expp,
            tc.tile_pool(name="yp", bufs=4) as yp,
            tc.tile_pool(name="smallp", bufs=8) as smallp,
        ):
            # ---- constants (one DMA) + x loads spread across DMA queues ----
            cb = consts.tile([128, CBLOB], f32r)
            nc.scalar.dma_start(cb[:], cblob_d[:, :].bitcast(f32r))
            # views into the blob (layout must match _pack_consts)
            wq4s = lambda kt: cb[:, 128 * kt : 128 * (kt + 1)]
            wk4s = lambda kt: cb[:, 256 + 128 * kt : 256 + 128 * (kt + 1)]
            wvs = lambda kt: cb[:, 512 + CH2 * kt : 512 + CH2 * (kt + 1)]
            bq4s = cb[:, 1028:1029].bitcast(f32)
            bk4s = cb[:, 1029:1030].bitcast(f32)
            bvs = cb[0:1, 1030 : 1030 + CH2]
            gs = cb[0:1, 1288:1290]
            oness = cb[0:1, 1290:1418]

            xts = xtp.tile([128, 2, N], f32r)
            xTr = xT.rearrange("(t p) n -> p t n", p=128)
            # Fine-grained chunks so compute starts on early columns while
            # later ones still stream; round-robin the three DMA queues.
            dmae = [nc.sync, nc.gpsimd, nc.sync]
            di = 0
            for s in range(N // 512):
                for kt in range(2):
                    dmae[di % len(dmae)].dma_start(
                        xts[:, kt, 512 * s : 512 * (s + 1)],
                        xTr[:, kt, 512 * s : 512 * (s + 1)].bitcast(f32r),
                    )
                    di += 1

            xr = xrp.tile([128, NQ // 128, CH], f32)
            nc.gpsimd.dma_start(xr[:], xres.rearrange("(t p) c -> p t c", p=128))

            qt4 = qkp.tile([128, NQ], f32r)
            kt4 = qkp.tile([128, N], f32r)
            vaug = vp.tile([128, MT, CH2], f32r)

            # ---- broadcasts (bias row, gamma) via K=1 outer-product matmuls ----
            with tc.tile_pool(name="psb", bufs=2, space="PSUM") as psb:
                pb = psb.tile([128, CH2], f32)
                nc.tensor.matmul(
                    pb[:],
                    lhsT=oness.bitcast(f32r),
                    rhs=bvs.bitcast(f32r),
                    start=True,
                    stop=True,
                )
                bvb = consts.tile([128, CH2], f32)
                nc.vector.tensor_copy(bvb[:], pb[:])

                pg = psb.tile([128, 2], f32)
                nc.tensor.matmul(
                    pg[:],
                    lhsT=oness.bitcast(f32r),
                    rhs=gs.bitcast(f32r),
                    start=True,
                    stop=True,
                )
                gb = consts.tile([128, 2], f32)
                nc.vector.tensor_copy(gb[:], pg[:])

            # ---- projections ----
            with (
                tc.tile_pool(name="psqk", bufs=2, space="PSUM") as psqk,
                tc.tile_pool(name="psv", bufs=4, space="PSUM") as psv,
            ):
                # qT4[32a+d, n] = q[n, d] (own half), replicated over a
                for t in range(NS):
                    pq = psqk.tile([128, 512], f32)
                    for kt in range(2):
                        nc.tensor.matmul(
                            pq[:],
                            lhsT=wq4s(kt).bitcast(f32r),
                            rhs=xts[:, kt, 512 * t : 512 * (t + 1)].bitcast(f32r),
                            start=(kt == 0),
                            stop=(kt == 1),
                        )
                    nc.vector.tensor_scalar_add(
                        qt4[:, 512 * t : 512 * (t + 1)], pq[:], bq4s
                    )
                # kT4 over the full batch
                for t in range(N // 512):
                    pk = psqk.tile([128, 512], f32)
                    for kt in range(2):
                        nc.tensor.matmul(
                            pk[:],
                            lhsT=wk4s(kt).bitcast(f32r),
                            rhs=xts[:, kt, 512 * t : 512 * (t + 1)].bitcast(f32r),
                            start=(kt == 0),
                            stop=(kt == 1),
                        )
                    nc.vector.tensor_scalar_add(
                        kt4[:, 512 * t : 512 * (t + 1)], pk[:], bk4s
                    )
                # v_aug[m, 0:256] = v natural; v_aug[m, 256] = 1 (bias col)
                for mt in range(MT):
                    pv = psv.tile([128, CH2], f32)
                    for kt in range(2):
                        nc.tensor.matmul(
                            pv[:],
                            lhsT=xts[:, kt, 128 * mt : 128 * (mt + 1)].bitcast(f32r),
                            rhs=wvs(kt).bitcast(f32r),
                            start=(kt == 0),
                            stop=(kt == 1),
                        )
                    nc.vector.tensor_tensor(
                        vaug[:, mt, :], pv[:], bvb[:], op=OP.add
                    )

            # ---- attention main loop ----
            # Groups of 2 key tiles; score PSUM double-buffered (2+2 banks)
            # so the PE can prefill the next group's scores while the
            # ScalarE exps the current one. Consecutive groups use disjoint
            # PE row-strip pairs so their packed matmuls overlap in the
            # array as well.
            NG2 = MT // 2  # 16 groups per n-slice
            with (
                tc.tile_pool(name="pss", bufs=2, space="PSUM") as pss,
                tc.tile_pool(name="psa", bufs=1, space="PSUM") as psa,
            ):
                def scores_mm(ns, g, s):
                    # scoresT[m, n] for 2 key tiles, row-packed (K=32 x2);
                    # alternating strip pairs let consecutive groups overlap
                    for i in range(2):
                        mt = 2 * g + i
                        st = (2 * g + i) % 4  # row strip
                        nc.tensor.matmul(
                            s[:, i, :],
                            lhsT=kt4[
                                32 * st : 32 * (st + 1), 128 * mt : 128 * (mt + 1)
                            ].bitcast(f32r),
                            rhs=qt4[
                                32 * st : 32 * (st + 1), 512 * ns : 512 * (ns + 1)
                            ].bitcast(f32r),
                            start=True,
                            stop=True,
                            tile_position=(32 * st, 0),
                        )

                groups = [(ns, g) for ns in range(NS) for g in range(NG2)]
                acc = None
                s_cur = pss.tile([128, 2, 512], f32)
                scores_mm(*groups[0], s_cur)
                for idx, (ns, g) in enumerate(groups):
                    if g == 0:
                        acc = psa.tile([128, 4, 512], f32)
                    e = expp.tile([128, 2, 512], f32r)
                    nc.scalar.activation(e[:], s_cur[:], AF.Exp)
                    # emit the next group's scores BEFORE this group's accum
                    # so the PE unblocks the ScalarE first
                    if idx + 1 < len(groups):
                        s_cur = pss.tile([128, 2, 512], f32)
                        scores_mm(*groups[idx + 1], s_cur)
                    # acc[n, :] += expT[m, n].T-as-weights @ v_aug[m, :]
                    for i in range(2):
                        mt = 2 * g + i
                        for j in range(4):
                            nc.tensor.matmul(
                                acc[:, j, 0:CH2],
                                lhsT=e[:, i, 128 * j : 128 * (j + 1)].bitcast(f32r),
                                rhs=vaug[:, mt, :].bitcast(f32r),
                                start=(g == 0 and i == 0),
                                stop=(g == NG2 - 1 and i == 1),
                            )
                    if g == NG2 - 1:
                        # normalize + gamma + residual, store
                        for j in range(4):
                            nt = 4 * ns + j
                            r = smallp.tile([128, 1], f32)
                            nc.vector.reciprocal(r[:], acc[:, j, CH : CH + 1])
                            rg = smallp.tile([128, 1], f32)
                            nc.vector.tensor_tensor(rg[:], r[:], gb[:, 0:1], op=OP.mult)
                            yt = yp.tile([128, CH], f32)
                            nc.vector.scalar_tensor_tensor(
                                yt[:],
                                acc[:, j, 0:CH],
                                rg[:, 0:1],
                                xr[:, nt, :],
                                op0=OP.mult,
                                op1=OP.add,
                            )
                            nc.sync.dma_start(y[128 * nt : 128 * (nt + 1), :], yt[:])

    nc.compile()
    _COMPILED["nc"] = nc
    return nc


def _pack_consts(Wq, bq, Wk, bk, Wv, bv, gamma):
    """Pack all small constants into one [128, CBLOB] blob.

    Layout (per partition p):
      [0:256)     Wq4 k-tiles: [wq4[p], wq4[p+128]]   (wq4 = tile(Wq, (1,4)))
      [256:512)   Wk4 k-tiles
      [512:1028)  Wv_aug k-tiles (CH2 = 258 each)
      [1028]      bq4[p];  [1029] bk4[p]
      partition 0 only:
      [1030:1288) bv_aug (bv ++ [1.0, 0.0])
      [1288:1290) gamma, 0
      [1290:1418) ones
    """
    Wq4 = np.tile(np.asarray(Wq, np.float32), (1, 4))  # [256, 128]
    Wk4 = np.tile(np.asarray(Wk, np.float32), (1, 4))
    bq4 = np.tile(np.asarray(bq, np.float32), 4)  # [128]
    bk4 = np.tile(np.asarray(bk, np.float32), 4)
    Wv_aug = np.zeros((CH, CH2), np.float32)
    Wv_aug[:, :CH] = np.asarray(Wv, np.float32)

    cb = np.zeros((128, CBLOB), np.float32)
    for kt in range(2):
        cb[:, 128 * kt : 128 * (kt + 1)] = Wq4[128 * kt : 128 * (kt + 1), :]
        cb[:, 256 + 128 * kt : 256 + 128 * (kt + 1)] = Wk4[128 * kt : 128 * (kt + 1)]
        cb[:, 512 + CH2 * kt : 512 + CH2 * (kt + 1)] = Wv_aug[
            128 * kt : 128 * (kt + 1), :
        ]
    cb[:, 1028] = bq4
    cb[:, 1029] = bk4
    cb[0, 1030 : 1030 + CH] = np.asarray(bv, np.float32)
    cb[0, 1030 + CH] = 1.0
    cb[0, 1288] = np.float32(np.asarray(gamma).reshape(()))
    cb[0, 1290:1418] = 1.0
    return cb


def _shard_inputs(x, Wq, bq, Wk, bk, Wv, bv, gamma):
    """Host-side prep: one input map per core."""
    xf = np.ascontiguousarray(x, dtype=np.float32).reshape(B, N, CH)
    cb = _pack_consts(Wq, bq, Wk, bk, Wv, bv, gamma)

    in_maps = []
    for c in range(N_CORES):
        b, h = divmod(c, 2)
        own = slice(h * NQ, (h + 1) * NQ)
        other = slice((1 - h) * NQ, (2 - h) * NQ)
        xT_b = xf[b].T  # [CH, N]
        xT_roll = np.ascontiguousarray(
            np.concatenate([xT_b[:, own], xT_b[:, other]], axis=1)
        )
        in_maps.append(
            {
                "xT": xT_roll,
                "xres": np.ascontiguousarray(xf[b, own]),
                "cblob": cb,
            }
        )
    return in_maps


def kernel(x, Wq, bq, Wk, bk, Wv, bv, gamma):
    from concourse.bass_utils import run_bass_kernel_spmd

    nc = _build()
    in_maps = _shard_inputs(x, Wq, bq, Wk, bk, Wv, bv, gamma)
    res = run_bass_kernel_spmd(nc, in_maps, core_ids=list(range(N_CORES)))
    out = np.empty((B, N, CH), np.float32)
    for c in range(N_CORES):
        b, h = divmod(c, 2)
        out[b, h * NQ : (h + 1) * NQ, :] = res.results[c]["y"]
    return out.reshape(x.shape)


# revision 22
# speedup vs baseline: 1.2834x; 1.2425x over previous
"""AttentionBlock Trainium2 kernel.

Reference computation (per batch b):
    xf = x[b].reshape(N, C);  N = 64*64 = 4096, C = 256, d = C//8 = 32
    q = xf @ Wq + bq; k = xf @ Wk + bk; v = xf @ Wv + bv
    out = softmax(q @ k.T) @ v
    y = gamma * out + xf

Sharding: 8 cores = 4 batches x 2 halves of the query rows. Each core
computes k/v for its full batch and attention for its 2048 query rows.

Per-core kernel layout choices:
  - Host passes xT (x[b] transposed, own query half rolled to the front) so
    all projection matmuls contract over channels on the partition dim.
  - q/k are projected with 4x-replicated weights (Wq tiled to [256,128]) so
    the d=32 contraction of the score matmul can be row-packed 4 ways
    (tile_position) and fill the whole 128x128 PE array.
  - Scores are computed TRANSPOSED (scoresT[m, n] = k[m].q[n]) so that after
    exp, the attention weights are already in the right layout to be the
    stationary operand of the attn@v matmul, with output in natural [n, c]
    layout - no transposes anywhere.
  - v is augmented with a ones column, so the attn@v accumulation also
    produces the softmax denominator (column 256) for free.
  - All matmuls use float32r (full-rate fp32 mode on the PE array).
"""

import numpy as np

CH = 256
DQK = 32
N = 4096  # H*W
NQ = 2048  # query rows per core
B = 4
N_CORES = 8
CH2 = CH + 2  # v augmented with [denominator-ones, pad] columns (fp32r needs even)
CBLOB = 1418  # packed constants blob width (see _pack_consts)

_COMPILED = {}


def _build():
    """Build + compile the single-program SPMD Bass kernel. Cached."""
    if "nc" in _COMPILED:
        return _COMPILED["nc"]

    import concourse.bass as bass
    import concourse.tile as tile
    from concourse import bacc, mybir

    f32 = mybir.dt.float32
    f32r = mybir.dt.float32r
    AF = mybir.ActivationFunctionType
    OP = mybir.AluOpType

    nc = bacc.Bacc(
        "TRN2",
        target_bir_lowering=False,
        debug=False,
        enable_asserts=True,
        num_devices=N_CORES,
    )

    # ---- I/O ----
    xT = nc.dram_tensor("xT", [CH, N], f32, kind="ExternalInput").ap()
    xres = nc.dram_tensor("xres", [NQ, CH], f32, kind="ExternalInput").ap()
    # all small constants packed into one blob (single DMA); see _pack_consts
    cblob_d = nc.dram_tensor("cblob", [128, CBLOB], f32, kind="ExternalInput").ap()
    y = nc.dram_tensor("y", [NQ, CH], f32, kind="ExternalOutput").ap()

    MT = N // 128  # 32 key tiles
    NS = NQ // 512  # 4 query slices
    NGRP = MT // 4  # 8 groups of 4 key tiles

    with tile.TileContext(nc) as tc:
        with (
            tc.tile_pool(name="consts", bufs=1) as consts,
            tc.tile_pool(name="xtp", bufs=1) as xtp,
            tc.tile_pool(name="qk", bufs=1) as qkp,
            tc.tile_pool(name="vp", bufs=1) as vp,
            tc.tile_pool(name="xrp", bufs=1) as xrp,
            tc.tile_pool(name="expp", bufs=3) as expp,
            tc.tile_pool(name="yp", bufs=2) as yp,
            tc.tile_pool(name="smallp", bufs=8) as smallp,
        ):
            # ---- constants (one DMA) + x loads spread across DMA queues ----
            cb = consts.tile([128, CBLOB], f32r)
            nc.scalar.dma_start(cb[:], cblob_d[:, :].bitcast(f32r))
            # views into the blob (layout must match _pack_consts)
            wq4s = lambda kt: cb[:, 128 * kt : 128 * (kt + 1)]
            wk4s = lambda kt: cb[:, 256 + 128 * kt : 256 + 128 * (kt + 1)]
            wvs = lambda kt: cb[:, 512 + CH2 * kt : 512 + CH2 * (kt + 1)]
            bq4s = cb[:, 1028:1029].bitcast(f32)
            bk4s = cb[:, 1029:1030].bitcast(f32)
            bvs = cb[0:1, 1030 : 1030 + CH2]
            gs = cb[0:1, 1288:1290]
            oness = cb[0:1, 1290:1418]

            xts = xtp.tile([128, 2, N], f32r)
            xTr = xT.rearrange("(t p) n -> p t n", p=128)
            # Each dma_start costs ~700ns of descriptor issue on its queue,
            # so use few descriptors: a small first chunk so compute starts
            # early, then progressively larger ones streaming behind.
            for lo, hi in [(0, 512), (512, 1024), (1024, 2048), (2048, 4096)]:
                nc.sync.dma_start(
                    xts[:, :, lo:hi], xTr[:, :, lo:hi].bitcast(f32r)
                )

            xr = xrp.tile([128, NQ // 128, CH], f32)
            nc.scalar.dma_start(xr[:], xres.rearrange("(t p) c -> p t c", p=128))

            qt4 = qkp.tile([128, NQ], f32r)
            kt4 = qkp.tile([128, N], f32r)
            vaug = vp.tile([128, MT, CH2], f32r)

            # ---- broadcasts (bias row, gamma) via K=1 outer-product matmuls ----
            with tc.tile_pool(name="psb", bufs=2, space="PSUM") as psb:
                pb = psb.tile([128, CH2], f32)
                nc.tensor.matmul(
                    pb[:],
                    lhsT=oness.bitcast(f32r),
                    rhs=bvs.bitcast(f32r),
                    start=True,
                    stop=True,
                )
                bvb = consts.tile([128, CH2], f32)
                nc.vector.tensor_copy(bvb[:], pb[:])

                pg = psb.tile([128, 2], f32)
                nc.tensor.matmul(
                    pg[:],
                    lhsT=oness.bitcast(f32r),
                    rhs=gs.bitcast(f32r),
                    start=True,
                    stop=True,
                )
                gb = consts.tile([128, 2], f32)
                nc.vector.tensor_copy(gb[:], pg[:])

            # ---- projections ----
            with (
                tc.tile_pool(name="psqk", bufs=2, space="PSUM") as psqk,
                tc.tile_pool(name="psv", bufs=4, space="PSUM") as psv,
            ):
                # qT4[32a+d, n] = q[n, d] (own half), replicated over a
                for t in range(NS):
                    pq = psqk.tile([128, 512], f32)
                    for kt in range(2):
                        nc.tensor.matmul(
                            pq[:],
                            lhsT=wq4s(kt).bitcast(f32r),
                            rhs=xts[:, kt, 512 * t : 512 * (t + 1)].bitcast(f32r),
                            start=(kt == 0),
                            stop=(kt == 1),
                        )
                    nc.vector.tensor_scalar_add(
                        qt4[:, 512 * t : 512 * (t + 1)], pq[:], bq4s
                    )
                # kT4 over the full batch
                for t in range(N // 512):
                    pk = psqk.tile([128, 512], f32)
                    for kt in range(2):
                        nc.tensor.matmul(
                            pk[:],
                            lhsT=wk4s(kt).bitcast(f32r),
                            rhs=xts[:, kt, 512 * t : 512 * (t + 1)].bitcast(f32r),
                            start=(kt == 0),
                            stop=(kt == 1),
                        )
                    nc.vector.tensor_scalar_add(
                        kt4[:, 512 * t : 512 * (t + 1)], pk[:], bk4s
                    )
                # v_aug[m, 0:256] = v natural; v_aug[m, 256] = 1 (bias col)
                for mt in range(MT):
                    pv = psv.tile([128, CH2], f32)
                    for kt in range(2):
                        nc.tensor.matmul(
                            pv[:],
                            lhsT=xts[:, kt, 128 * mt : 128 * (mt + 1)].bitcast(f32r),
                            rhs=wvs(kt).bitcast(f32r),
                            start=(kt == 0),
                            stop=(kt == 1),
                        )
                    nc.vector.tensor_tensor(
                        vaug[:, mt, :], pv[:], bvb[:], op=OP.add
                    )

            # ---- attention main loop ----
            # Groups of 2 key tiles; score PSUM double-buffered (2+2 banks)
            # so the PE can prefill the next group's scores while the
            # ScalarE exps the current one. Consecutive groups use disjoint
            # PE row-strip pairs so their packed matmuls overlap in the
            # array as well.
            NG2 = MT // 2  # 16 groups per n-slice
            with (
                tc.tile_pool(name="pss", bufs=2, space="PSUM") as pss,
                tc.tile_pool(name="psa", bufs=1, space="PSUM") as psa,
            ):
                def scores_mm(ns, g, s):
                    # scoresT[m, n], one K=128 matmul per key tile: the 4x
                    # replication of q/k means the 128-deep contraction sums
                    # 4 copies of q.k (Wq is pre-scaled by 1/4 on the host).
                    for i in range(2):
                        mt = 2 * g + i
                        nc.tensor.matmul(
                            s[:, i, :],
                            lhsT=kt4[:, 128 * mt : 128 * (mt + 1)].bitcast(f32r),
                            rhs=qt4[:, 512 * ns : 512 * (ns + 1)].bitcast(f32r),
                            start=True,
                            stop=True,
                        )

                groups = [(ns, g) for ns in range(NS) for g in range(NG2)]
                acc = None
                # two groups of scores in flight ahead of the exp stream
                s_tiles = {}
                for la in range(2):
                    s_tiles[la] = pss.tile([128, 2, 512], f32, tag="s", name=f"sc{la}")
                    scores_mm(*groups[la], s_tiles[la])
                for idx, (ns, g) in enumerate(groups):
                    if g == 0:
                        acc = psa.tile([128, 4, 512], f32)
                    e = expp.tile([128, 2, 512], f32r)
                    nc.scalar.activation(e[:], s_tiles.pop(idx % 2)[:], AF.Exp)
                    # keep the scores pipeline 2 deep before emitting accums
                    if idx + 2 < len(groups):
                        s_tiles[idx % 2] = pss.tile([128, 2, 512], f32, tag="s", name=f"sc{idx}")
                        scores_mm(*groups[idx + 2], s_tiles[idx % 2])
                    # acc[n, :] += expT[m, n].T-as-weights @ v_aug[m, :]
                    for i in range(2):
                        mt = 2 * g + i
                        for j in range(4):
                            nc.tensor.matmul(
                                acc[:, j, 0:CH2],
                                lhsT=e[:, i, 128 * j : 128 * (j + 1)].bitcast(f32r),
                                rhs=vaug[:, mt, :].bitcast(f32r),
                                start=(g == 0 and i == 0),
                                stop=(g == NG2 - 1 and i == 1),
                            )
                    if g == NG2 - 1:
                        # evacuate acc quickly (one copy) so the next slice's
                        # accumulation isn't blocked on the normalize chain
                        accs = yp.tile([128, 4, CH2], f32, tag="accs")
                        nc.vector.tensor_copy(accs[:], acc[:, :, 0:CH2])
                        yt = yp.tile([128, 4, CH], f32, tag="yt")
                        for j in range(4):
                            nt = 4 * ns + j
                            r = smallp.tile([128, 1], f32)
                            nc.vector.reciprocal(r[:], accs[:, j, CH : CH + 1])
                            rg = smallp.tile([128, 1], f32)
                            nc.vector.tensor_tensor(rg[:], r[:], gb[:, 0:1], op=OP.mult)
                            nc.vector.scalar_tensor_tensor(
                                yt[:, j, :],
                                accs[:, j, 0:CH],
                                rg[:, 0:1],
                                xr[:, nt, :],
                                op0=OP.mult,
                                op1=OP.add,
                            )
                        nc.sync.dma_start(
                            y.rearrange("(t p) c -> p t c", p=128)[
                                :, 4 * ns : 4 * (ns + 1), :
                            ],
                            yt[:],
                        )

    nc.compile()
    _COMPILED["nc"] = nc
    return nc


def _pack_consts(Wq, bq, Wk, bk, Wv, bv, gamma):
    """Pack all small constants into one [128, CBLOB] blob.

    Layout (per partition p):
      [0:256)     Wq4 k-tiles: [wq4[p], wq4[p+128]]   (wq4 = tile(Wq, (1,4)))
      [256:512)   Wk4 k-tiles
      [512:1028)  Wv_aug k-tiles (CH2 = 258 each)
      [1028]      bq4[p];  [1029] bk4[p]
      partition 0 only:
      [1030:1288) bv_aug (bv ++ [1.0, 0.0])
      [1288:1290) gamma, 0
      [1290:1418) ones
    """
    # Wq/bq scaled by 1/4: the K=128 score matmul sums over the 4 replicas
    Wq4 = np.tile(np.asarray(Wq, np.float32) * 0.25, (1, 4))  # [256, 128]
    Wk4 = np.tile(np.asarray(Wk, np.float32), (1, 4))
    bq4 = np.tile(np.asarray(bq, np.float32) * 0.25, 4)  # [128]
    bk4 = np.tile(np.asarray(bk, np.float32), 4)
    Wv_aug = np.zeros((CH, CH2), np.float32)
    Wv_aug[:, :CH] = np.asarray(Wv, np.float32)

    cb = np.zeros((128, CBLOB), np.float32)
    for kt in range(2):
        cb[:, 128 * kt : 128 * (kt + 1)] = Wq4[128 * kt : 128 * (kt + 1), :]
        cb[:, 256 + 128 * kt : 256 + 128 * (kt + 1)] = Wk4[128 * kt : 128 * (kt + 1)]
        cb[:, 512 + CH2 * kt : 512 + CH2 * (kt + 1)] = Wv_aug[
            128 * kt : 128 * (kt + 1), :
        ]
    cb[:, 1028] = bq4
    cb[:, 1029] = bk4
    cb[0, 1030 : 1030 + CH] = np.asarray(bv, np.float32)
    cb[0, 1030 + CH] = 1.0
    cb[0, 1288] = np.float32(np.asarray(gamma).reshape(()))
    cb[0, 1290:1418] = 1.0
    return cb


def _shard_inputs(x, Wq, bq, Wk, bk, Wv, bv, gamma):
    """Host-side prep: one input map per core."""
    xf = np.ascontiguousarray(x, dtype=np.float32).reshape(B, N, CH)
    cb = _pack_consts(Wq, bq, Wk, bk, Wv, bv, gamma)

    in_maps = []
    for c in range(N_CORES):
        b, h = divmod(c, 2)
        own = slice(h * NQ, (h + 1) * NQ)
        other = slice((1 - h) * NQ, (2 - h) * NQ)
        xT_b = xf[b].T  # [CH, N]
        xT_roll = np.ascontiguousarray(
            np.concatenate([xT_b[:, own], xT_b[:, other]], axis=1)
        )
        in_maps.append(
            {
                "xT": xT_roll,
                "xres": np.ascontiguousarray(xf[b, own]),
                "cblob": cb,
            }
        )
    return in_maps


def kernel(x, Wq, bq, Wk, bk, Wv, bv, gamma):
    from concourse.bass_utils import run_bass_kernel_spmd

    nc = _build()
    in_maps = _shard_inputs(x, Wq, bq, Wk, bk, Wv, bv, gamma)
    res = run_bass_kernel_spmd(nc, in_maps, core_ids=list(range(N_CORES)))
    out = np.empty((B, N, CH), np.float32)
    for c in range(N_CORES):
        b, h = divmod(c, 2)
        out[b, h * NQ : (h + 1) * NQ, :] = res.results[c]["y"]
    return out.reshape(x.shape)


# revision 23
# speedup vs baseline: 1.2847x; 1.0011x over previous
"""AttentionBlock Trainium2 kernel.

Reference computation (per batch b):
    xf = x[b].reshape(N, C);  N = 64*64 = 4096, C = 256, d = C//8 = 32
    q = xf @ Wq + bq; k = xf @ Wk + bk; v = xf @ Wv + bv
    out = softmax(q @ k.T) @ v
    y = gamma * out + xf

Sharding: 8 cores = 4 batches x 2 halves of the query rows. Each core
computes k/v for its full batch and attention for its 2048 query rows.

Per-core kernel layout choices:
  - Host passes xT (x[b] transposed, own query half rolled to the front) so
    all projection matmuls contract over channels on the partition dim.
  - q/k are projected with 4x-replicated weights (Wq tiled to [256,128]) so
    the d=32 contraction of the score matmul can be row-packed 4 ways
    (tile_position) and fill the whole 128x128 PE array.
  - Scores are computed TRANSPOSED (scoresT[m, n] = k[m].q[n]) so that after
    exp, the attention weights are already in the right layout to be the
    stationary operand of the attn@v matmul, with output in natural [n, c]
    layout - no transposes anywhere.
  - v is augmented with a ones column, so the attn@v accumulation also
    produces the softmax denominator (column 256) for free.
  - All matmuls use float32r (full-rate fp32 mode on the PE array).
"""

import numpy as np

CH = 256
DQK = 32
N = 4096  # H*W
NQ = 2048  # query rows per core
B = 4
N_CORES = 8
CH2 = CH + 2  # v augmented with [denominator-ones, pad] columns (fp32r needs even)
CBLOB = 1418  # packed constants blob width (see _pack_consts)

_COMPILED = {}


def _build():
    """Build + compile the single-program SPMD Bass kernel. Cached."""
    if "nc" in _COMPILED:
        return _COMPILED["nc"]

    import concourse.bass as bass
    import concourse.tile as tile
    from concourse import bacc, mybir

    f32 = mybir.dt.float32
    f32r = mybir.dt.float32r
    AF = mybir.ActivationFunctionType
    OP = mybir.AluOpType

    nc = bacc.Bacc(
        "TRN2",
        target_bir_lowering=False,
        debug=False,
        enable_asserts=True,
        num_devices=N_CORES,
    )

    # ---- I/O ----
    xT = nc.dram_tensor("xT", [CH, N], f32, kind="ExternalInput").ap()
    xres = nc.dram_tensor("xres", [NQ, CH], f32, kind="ExternalInput").ap()
    # all small constants packed into one blob (single DMA); see _pack_consts
    cblob_d = nc.dram_tensor("cblob", [128, CBLOB], f32, kind="ExternalInput").ap()
    y = nc.dram_tensor("y", [NQ, CH], f32, kind="ExternalOutput").ap()

    MT = N // 128  # 32 key tiles
    NS = NQ // 512  # 4 query slices
    NGRP = MT // 4  # 8 groups of 4 key tiles

    with tile.TileContext(nc) as tc:
        with (
            tc.tile_pool(name="consts", bufs=1) as consts,
            tc.tile_pool(name="xtp", bufs=1) as xtp,
            tc.tile_pool(name="qk", bufs=1) as qkp,
            tc.tile_pool(name="vp", bufs=1) as vp,
            tc.tile_pool(name="xrp", bufs=1) as xrp,
            tc.tile_pool(name="expp", bufs=3) as expp,
            tc.tile_pool(name="yp", bufs=2) as yp,
            tc.tile_pool(name="smallp", bufs=8) as smallp,
        ):
            # ---- constants (one DMA) + x loads spread across DMA queues ----
            cb = consts.tile([128, CBLOB], f32r)
            nc.sync.dma_start(cb[:], cblob_d[:, :].bitcast(f32r))
            # views into the blob (layout must match _pack_consts)
            wq4s = lambda kt: cb[:, 128 * kt : 128 * (kt + 1)]
            wk4s = lambda kt: cb[:, 256 + 128 * kt : 256 + 128 * (kt + 1)]
            wvs = lambda kt: cb[:, 512 + CH2 * kt : 512 + CH2 * (kt + 1)]
            bq4s = cb[:, 1028:1029].bitcast(f32)
            bk4s = cb[:, 1029:1030].bitcast(f32)
            bvs = cb[0:1, 1030 : 1030 + CH2]
            gs = cb[0:1, 1288:1290]
            oness = cb[0:1, 1290:1418]

            xts = xtp.tile([128, 2, N], f32r)
            xTr = xT.rearrange("(t p) n -> p t n", p=128)
            # Each dma_start costs ~700ns of descriptor issue on its queue,
            # so use few descriptors: small first chunks (compute starts
            # early) on one queue, the big tail chunk in parallel on the
            # other.
            for lo, hi in [(0, 512), (512, 1024), (1024, 2048)]:
                nc.sync.dma_start(
                    xts[:, :, lo:hi], xTr[:, :, lo:hi].bitcast(f32r)
                )
            nc.scalar.dma_start(
                xts[:, :, 2048:4096], xTr[:, :, 2048:4096].bitcast(f32r)
            )

            xr = xrp.tile([128, NQ // 128, CH], f32)
            nc.scalar.dma_start(xr[:], xres.rearrange("(t p) c -> p t c", p=128))

            qt4 = qkp.tile([128, NQ], f32r)
            kt4 = qkp.tile([128, N], f32r)
            vaug = vp.tile([128, MT, CH2], f32r)

            # ---- broadcasts (bias row, gamma) via K=1 outer-product matmuls
            # plus dummy matmuls on the constant blob: they only depend on
            # the (tiny, early) cb DMA and warm the PE clock gate (HAM) so
            # the real projections run at 2.4 GHz ----
            with (
                tc.tile_pool(name="psb", bufs=2, space="PSUM") as psb,
                tc.tile_pool(name="warmp", bufs=2, space="PSUM") as warmp,
            ):
                warm_sink = consts.tile([128, 1], f32)
                for w in range(10):
                    wt = warmp.tile([128, 512], f32, tag="warm", name=f"warm{w}")
                    nc.tensor.matmul(
                        wt[:],
                        lhsT=cb[:, 0:128],
                        rhs=cb[:, 512:1024],
                        start=True,
                        stop=True,
                    )
                    if w == 9:
                        # keep the chain observable so it isn't dead-code
                        nc.vector.tensor_reduce(
                            warm_sink[:], wt[:], axis=mybir.AxisListType.X,
                            op=OP.max,
                        )
                pb = psb.tile([128, CH2], f32)
                nc.tensor.matmul(
                    pb[:],
                    lhsT=oness.bitcast(f32r),
                    rhs=bvs.bitcast(f32r),
                    start=True,
                    stop=True,
                )
                bvb = consts.tile([128, CH2], f32)
                nc.vector.tensor_copy(bvb[:], pb[:])

                pg = psb.tile([128, 2], f32)
                nc.tensor.matmul(
                    pg[:],
                    lhsT=oness.bitcast(f32r),
                    rhs=gs.bitcast(f32r),
                    start=True,
                    stop=True,
                )
                gb = consts.tile([128, 2], f32)
                nc.vector.tensor_copy(gb[:], pg[:])

            # ---- projections ----
            with (
                tc.tile_pool(name="psqk", bufs=2, space="PSUM") as psqk,
                tc.tile_pool(name="psv", bufs=4, space="PSUM") as psv,
            ):
                # qT4[32a+d, n] = q[n, d] (own half), replicated over a
                for t in range(NS):
                    pq = psqk.tile([128, 512], f32)
                    for kt in range(2):
                        nc.tensor.matmul(
                            pq[:],
                            lhsT=wq4s(kt).bitcast(f32r),
                            rhs=xts[:, kt, 512 * t : 512 * (t + 1)].bitcast(f32r),
                            start=(kt == 0),
                            stop=(kt == 1),
                        )
                    nc.scalar.activation(
                        qt4[:, 512 * t : 512 * (t + 1)], pq[:],
                        AF.Identity, bias=bq4s,
                    )
                # kT4 over the full batch
                for t in range(N // 512):
                    pk = psqk.tile([128, 512], f32)
                    for kt in range(2):
                        nc.tensor.matmul(
                            pk[:],
                            lhsT=wk4s(kt).bitcast(f32r),
                            rhs=xts[:, kt, 512 * t : 512 * (t + 1)].bitcast(f32r),
                            start=(kt == 0),
                            stop=(kt == 1),
                        )
                    nc.scalar.activation(
                        kt4[:, 512 * t : 512 * (t + 1)], pk[:],
                        AF.Identity, bias=bk4s,
                    )
                # v_aug[m, 0:256] = v natural; v_aug[m, 256] = 1 (bias col)
                for mt in range(MT):
                    pv = psv.tile([128, CH2], f32)
                    for kt in range(2):
                        nc.tensor.matmul(
                            pv[:],
                            lhsT=xts[:, kt, 128 * mt : 128 * (mt + 1)].bitcast(f32r),
                            rhs=wvs(kt).bitcast(f32r),
                            start=(kt == 0),
                            stop=(kt == 1),
                        )
                    nc.vector.tensor_tensor(
                        vaug[:, mt, :], pv[:], bvb[:], op=OP.add
                    )

            # ---- attention main loop ----
            # Groups of 2 key tiles; score PSUM double-buffered (2+2 banks)
            # so the PE can prefill the next group's scores while the
            # ScalarE exps the current one. Consecutive groups use disjoint
            # PE row-strip pairs so their packed matmuls overlap in the
            # array as well.
            NG2 = MT // 2  # 16 groups per n-slice
            with (
                tc.tile_pool(name="pss", bufs=2, space="PSUM") as pss,
                tc.tile_pool(name="psa", bufs=1, space="PSUM") as psa,
            ):
                def scores_mm(ns, g, s):
                    # scoresT[m, n], one K=128 matmul per key tile: the 4x
                    # replication of q/k means the 128-deep contraction sums
                    # 4 copies of q.k (Wq is pre-scaled by 1/4 on the host).
                    for i in range(2):
                        mt = 2 * g + i
                        nc.tensor.matmul(
                            s[:, i, :],
                            lhsT=kt4[:, 128 * mt : 128 * (mt + 1)].bitcast(f32r),
                            rhs=qt4[:, 512 * ns : 512 * (ns + 1)].bitcast(f32r),
                            start=True,
                            stop=True,
                        )

                groups = [(ns, g) for ns in range(NS) for g in range(NG2)]
                acc = None
                # two groups of scores in flight ahead of the exp stream
                s_tiles = {}
                for la in range(2):
                    s_tiles[la] = pss.tile([128, 2, 512], f32, tag="s", name=f"sc{la}")
                    scores_mm(*groups[la], s_tiles[la])
                for idx, (ns, g) in enumerate(groups):
                    if g == 0:
                        acc = psa.tile([128, 4, 512], f32)
                    e = expp.tile([128, 2, 512], f32r)
                    nc.scalar.activation(e[:], s_tiles.pop(idx % 2)[:], AF.Exp)
                    # keep the scores pipeline 2 deep before emitting accums
                    if idx + 2 < len(groups):
                        s_tiles[idx % 2] = pss.tile([128, 2, 512], f32, tag="s", name=f"sc{idx}")
                        scores_mm(*groups[idx + 2], s_tiles[idx % 2])
                    # acc[n, :] += expT[m, n].T-as-weights @ v_aug[m, :]
                    for i in range(2):
                        mt = 2 * g + i
                        for j in range(4):
                            nc.tensor.matmul(
                                acc[:, j, 0:CH2],
                                lhsT=e[:, i, 128 * j : 128 * (j + 1)].bitcast(f32r),
                                rhs=vaug[:, mt, :].bitcast(f32r),
                                start=(g == 0 and i == 0),
                                stop=(g == NG2 - 1 and i == 1),
                            )
                    if g == NG2 - 1:
                        # evacuate acc quickly (one copy) so the next slice's
                        # accumulation isn't blocked on the normalize chain
                        accs = yp.tile([128, 4, CH2], f32, tag="accs")
                        nc.vector.tensor_copy(accs[:], acc[:, :, 0:CH2])
                        yt = yp.tile([128, 4, CH], f32, tag="yt")
                        for j in range(4):
                            nt = 4 * ns + j
                            r = smallp.tile([128, 1], f32)
                            nc.vector.reciprocal(r[:], accs[:, j, CH : CH + 1])
                            rg = smallp.tile([128, 1], f32)
                            nc.vector.tensor_tensor(rg[:], r[:], gb[:, 0:1], op=OP.mult)
                            nc.vector.scalar_tensor_tensor(
                                yt[:, j, :],
                                accs[:, j, 0:CH],
                                rg[:, 0:1],
                                xr[:, nt, :],
                                op0=OP.mult,
                                op1=OP.add,
                            )
                        nc.sync.dma_start(
                            y.rearrange("(t p) c -> p t c", p=128)[
                                :, 4 * ns : 4 * (ns + 1), :
                            ],
                            yt[:],
                        )

    nc.compile()
    _COMPILED["nc"] = nc
    return nc


def _pack_consts(Wq, bq, Wk, bk, Wv, bv, gamma):
    """Pack all small constants into one [128, CBLOB] blob.

    Layout (per partition p):
      [0:256)     Wq4 k-tiles: [wq4[p], wq4[p+128]]   (wq4 = tile(Wq, (1,4)))
      [256:512)   Wk4 k-tiles
      [512:1028)  Wv_aug k-tiles (CH2 = 258 each)
      [1028]      bq4[p];  [1029] bk4[p]
      partition 0 only:
      [1030:1288) bv_aug (bv ++ [1.0, 0.0])
      [1288:1290) gamma, 0
      [1290:1418) ones
    """
    # Wq/bq scaled by 1/4: the K=128 score matmul sums over the 4 replicas
    Wq4 = np.tile(np.asarray(Wq, np.float32) * 0.25, (1, 4))  # [256, 128]
    Wk4 = np.tile(np.asarray(Wk, np.float32), (1, 4))
    bq4 = np.tile(np.asarray(bq, np.float32) * 0.25, 4)  # [128]
    bk4 = np.tile(np.asarray(bk, np.float32), 4)
    Wv_aug = np.zeros((CH, CH2), np.float32)
    Wv_aug[:, :CH] = np.asarray(Wv, np.float32)

    cb = np.zeros((128, CBLOB), np.float32)
    for kt in range(2):
        cb[:, 128 * kt : 128 * (kt + 1)] = Wq4[128 * kt : 128 * (kt + 1), :]
        cb[:, 256 + 128 * kt : 256 + 128 * (kt + 1)] = Wk4[128 * kt : 128 * (kt + 1)]
        cb[:, 512 + CH2 * kt : 512 + CH2 * (kt + 1)] = Wv_aug[
            128 * kt : 128 * (kt + 1), :
        ]
    cb[:, 1028] = bq4
    cb[:, 1029] = bk4
    cb[0, 1030 : 1030 + CH] = np.asarray(bv, np.float32)
    cb[0, 1030 + CH] = 1.0
    cb[0, 1288] = np.float32(np.asarray(gamma).reshape(()))
    cb[0, 1290:1418] = 1.0
    return cb


def _shard_inputs(x, Wq, bq, Wk, bk, Wv, bv, gamma):
    """Host-side prep: one input map per core."""
    xf = np.ascontiguousarray(x, dtype=np.float32).reshape(B, N, CH)
    cb = _pack_consts(Wq, bq, Wk, bk, Wv, bv, gamma)

    in_maps = []
    for c in range(N_CORES):
        b, h = divmod(c, 2)
        own = slice(h * NQ, (h + 1) * NQ)
        other = slice((1 - h) * NQ, (2 - h) * NQ)
        xT_b = xf[b].T  # [CH, N]
        xT_roll = np.ascontiguousarray(
            np.concatenate([xT_b[:, own], xT_b[:, other]], axis=1)
        )
        in_maps.append(
            {
                "xT": xT_roll,
                "xres": np.ascontiguousarray(xf[b, own]),
                "cblob": cb,
            }
        )
    return in_maps


def kernel(x, Wq, bq, Wk, bk, Wv, bv, gamma):
    from concourse.bass_utils import run_bass_kernel_spmd

    nc = _build()
    in_maps = _shard_inputs(x, Wq, bq, Wk, bk, Wv, bv, gamma)
    res = run_bass_kernel_spmd(nc, in_maps, core_ids=list(range(N_CORES)))
    out = np.empty((B, N, CH), np.float32)
    for c in range(N_CORES):
        b, h = divmod(c, 2)
        out[b, h * NQ : (h + 1) * NQ, :] = res.results[c]["y"]
    return out.reshape(x.shape)
